# revision 1
# baseline (speedup 1.0000x reference)
"""Trainium2 Bass kernel for nn_ChannelAttention_56573309224430.

Sharding: data-parallel over batch B=8 across the 8 NeuronCores (per the
sharding hint); each core computes all H=8 heads for its batch element and
returns its [L, H, E] slice; host stacks slices into the full output.

On-core algorithm, in token-major permuted coordinates l' = token*16 + var
(the permutation is free: it is just a DMA access pattern on load/store):
  * the mask (s%64 > l%64) becomes block-causal over 16-wide token groups,
    so score tile (a, c) is only computed for a <= c (~half the work);
  * the same/diff-variate bias is 16-periodic: rank-17 constant rows
    appended as a small augmented matmul (K=24) accumulated onto PSUM;
  * the diagonal tile's triangular mask is rank-7: folded into the same
    augmented matmul via -30000 rows (exp -> exact 0);
  * softmax denominator comes from a ones-column appended to V;
  * RoPE (first 32 dims, interleaved pairs) applied on-chip with
    host-precomputed cos/sin tables (in permuted order).
Everything streams through fp16 matmul inputs (PE full rate + DMA-xbar
transposes); PSUM accumulation is fp32.
"""

import numpy as np

B, L, H, E = 8, 1024, 8, 64
NT, NV = 64, 16            # n_tokens, n_vars
P = 128                    # partitions
NTILE = L // P             # 8 tiles of 128 positions
SCALE = 1.0 / np.sqrt(E)   # 0.125
NEG = -30000.0             # large negative, fp16-representable
ROPE_BASE = 10000.0
PH = E // 2                # 32 rotary dims
NF = PH // 2               # 16 frequencies
NAUG = 17                  # 16 bias rows + 1 diff row (fused into QK)
KF = E + NAUG              # fused contraction depth 81

_CACHE = {}


# ----------------------------------------------------------------------------
# host-side constant construction
# ----------------------------------------------------------------------------

def _perm_pos():
    """pos(l') for token-major order l' = t*16 + v -> original pos v*64 + t."""
    lp = np.arange(L)
    return (lp % NV) * NT + (lp // NV)


def _rope_tables():
    """Full-width multiplier tables in permuted order, signs folded in.

    cosF[p, n, h, e]: cos(pos*theta) on e<32 (pair-repeated), 1.0 on e>=32.
    sinF[p, n, h, 2j] = -sin(pos*theta_j), sinF[.., 2j+1] = +sin, 0 on e>=32.
    rope: out = x * cosF + swap_pairs(x) * sinF  (swap over full width; the
    e>=32 half is multiplied by 0).
    """
    pos = _perm_pos().astype(np.float64)                       # [1024]
    theta = ROPE_BASE ** (-(np.arange(NF) / NF))               # [16]
    ang = pos[:, None] * theta[None, :]                        # [1024, 16]
    cosF = np.ones((L, E))
    cosF[:, 0:PH] = np.repeat(np.cos(ang), 2, axis=1)
    sinF = np.zeros((L, E))
    sinF[:, 0:PH:2] = -np.sin(ang)
    sinF[:, 1:PH:2] = np.sin(ang)

    def lay(tab):
        t = tab.reshape(NTILE, P, 1, E).transpose(1, 0, 2, 3)
        return np.ascontiguousarray(
            np.broadcast_to(t, (P, NTILE, H, E))).astype(np.float16)
    return lay(cosF), lay(sinF)


def _aug_tables(bias_emb):
    """kaugf [8, 17, 1024], qaugf [17, 1024], kneg7/qnegd7 [7, 128] (fp16)."""
    bdiff = bias_emb[0].astype(np.float64)   # [H]
    bsame = bias_emb[1].astype(np.float64)
    db = bsame - bdiff
    sp = np.arange(L)
    res16 = sp % 16                          # variate id in permuted order

    kaugf = np.zeros((H, NAUG, L))
    for h in range(H):
        for r in range(16):
            kaugf[h, r] = db[h] * (res16 == r)
        kaugf[h, 16] = bdiff[h]
    qaugf = np.zeros((NAUG, L))
    for r in range(16):
        qaugf[r] = (res16 == r)
    qaugf[16] = 1.0

    c = np.arange(P)
    kneg7 = np.zeros((7, P))
    qnegd7 = np.zeros((7, P))
    for gi in range(7):
        kneg7[gi] = NEG * (c // 16 == gi + 1)
        qnegd7[gi] = (c // 16 <= gi)
    return (kaugf.astype(np.float16), qaugf.astype(np.float16),
            kneg7.astype(np.float16), qnegd7.astype(np.float16))


# ----------------------------------------------------------------------------
# device program
# ----------------------------------------------------------------------------

def _emit(tc, aps):
    import concourse.bass as bass
    from concourse import mybir

    nc = tc.nc
    f32 = mybir.dt.float32
    f16 = mybir.dt.float16
    Exp = mybir.ActivationFunctionType.Exp

    def perm_tile(dram_ap, n, h=None):
        """Token-major view of DRAM [L, H, E]: tile n -> [(tr v)=128, h, e]."""
        t = dram_ap.tensor
        off = dram_ap.offset + n * 8 * (H * E)
        if h is None:
            ap = [[H * E, 8], [NT * H * E, NV], [E, H], [1, E]]
        else:
            off += h * E
            ap = [[H * E, 8], [NT * H * E, NV], [1, E]]
        return bass.AP(tensor=t, offset=off, ap=ap)

    q_d, k_d, v_d, out_d = aps["queries"], aps["keys"], aps["values"], aps["out"]

    from contextlib import ExitStack
    with ExitStack() as ctx:
        consts = ctx.enter_context(tc.tile_pool(name="consts", bufs=1))
        loadp = ctx.enter_context(tc.tile_pool(name="loadp", bufs=4))
        ropep = ctx.enter_context(tc.tile_pool(name="ropep", bufs=8))
        tmpp = ctx.enter_context(tc.tile_pool(name="tmpp", bufs=4))
        tposed = ctx.enter_context(tc.tile_pool(name="tposed", bufs=1))
        epool = ctx.enter_context(tc.tile_pool(name="epool", bufs=2))
        onorm = ctx.enter_context(tc.tile_pool(name="onorm", bufs=8))

        # ---- constants ----
        cos_sb = consts.tile([P, NTILE, H, E], f16)
        nc.scalar.dma_start(cos_sb, aps["cosF"])
        sin_sb = consts.tile([P, NTILE, H, E], f16)
        nc.scalar.dma_start(sin_sb, aps["sinF"])
        kneg_sb = consts.tile([7, P], f16)
        nc.sync.dma_start(kneg_sb, aps["kneg7"])
        qnegd_sb = consts.tile([7, P], f16)
        nc.sync.dma_start(qnegd_sb, aps["qnegd7"])
        ident_sb = consts.tile([P, P], f16)
        nc.sync.dma_start(ident_sb, aps["ident"])

        # fused q/k tiles: rows 0:64 = transposed rope'd data, 64:81 = aug rows
        qTA = [tposed.tile([KF, L], f16, name=f"qTA{h}") for h in range(H)]
        kTA = [tposed.tile([KF, L], f16, name=f"kTA{h}") for h in range(H)]
        for h in range(H):
            nc.scalar.dma_start(kTA[h][E:KF, :], aps["kaugf"][h])
            nc.scalar.dma_start(qTA[h][E:KF, :], aps["qaugf"])

        # ---- load + rope (q on DVE, k on GPSIMD), fp32 -> fp16 ----
        t16s = {}
        for n in range(NTILE):
            for which, src_d, eng in (("q", q_d, nc.vector), ("k", k_d, nc.gpsimd)):
                t32 = loadp.tile([P, H, E], f32, name=f"{which}32")
                (nc.sync if which == "q" else nc.scalar).dma_start(
                    t32, perm_tile(src_d, n))
                t16 = ropep.tile([P, H, E], f16, name=f"{which}16")
                m1 = tmpp.tile([P, H, E], f16, name=f"m1{which}")
                fl = lambda ap: ap.rearrange("p h e -> p (h e)")
                pr = lambda ap: fl(ap).rearrange("p (j i) -> p j i", i=2)
                eng.tensor_mul(fl(m1), fl(t32), fl(cos_sb[:, n]))
                m2 = tmpp.tile([P, H, E], f16, name=f"m2{which}")
                t32p, m2p, sinp = pr(t32), pr(m2), pr(sin_sb[:, n])
                eng.tensor_mul(m2p[:, :, 0], t32p[:, :, 1], sinp[:, :, 0])
                eng.tensor_mul(m2p[:, :, 1], t32p[:, :, 0], sinp[:, :, 1])
                eng.tensor_add(fl(t16), fl(m1), fl(m2))
                t16s[(which, n)] = t16

        # V' = [V | 1]: fp32 HWDGE load + gpsimd cast-copy into 66-stride
        v16 = consts.tile([P, NTILE, H, 66], f16)
        nc.vector.memset(v16[:, :, :, 64], 1.0)
        for a in range(NTILE):
            v32 = loadp.tile([P, H, E], f32, name="v32")
            nc.gpsimd.dma_start(v32, perm_tile(v_d, a))
            nc.gpsimd.tensor_copy(v16[:, a, :, 0:64], v32)

        # ---- attention with PE-transposes as inter-phase filler ----
        psc = ctx.enter_context(tc.tile_pool(name="psc", bufs=2, space="PSUM"))
        pout = ctx.enter_context(tc.tile_pool(name="pout", bufs=2, space="PSUM"))
        ptp = ctx.enter_context(tc.tile_pool(name="ptp", bufs=2, space="PSUM"))

        from concourse.tile_rust import add_dep_helper

        # PE warm-up: real matmuls (transpose-mode does not count for the HAM
        # activity window) to reach 2.4 GHz before the first QK matmul.
        wsb = consts.tile([P, 512], f16, name="wsb")
        nc.gpsimd.memset(wsb, 0.0)
        warm_ps = pout.tile([P, 512], f32, tag="oc", name="warm_ps")
        warm_last = None
        for _ in range(20):
            warm_last = nc.tensor.matmul(
                warm_ps, wsb[0:P, 0:P], wsb, start=True, stop=True)

        def emit_tp(which, h, first=False):
            dstT = qTA if which == "q" else kTA
            tp16 = ptp.tile([E, L], f16, tag="tp", name="tp16")
            # garbage matmul into the tile (overwritten by the transposes):
            # real-matmul PE activity so the HAM window never sees idle.
            g = nc.tensor.matmul(tp16.bitcast(f32), wsb[0:P, 0:E], wsb,
                                 start=True, stop=True, skip_group_check=True)
            if first and warm_last is not None:
                add_dep_helper(warm_last.ins, g.ins, sync=False,
                               reason="warmup before transposes")
            for n in range(NTILE):
                flat = t16s[(which, n)].rearrange("p h e -> p (h e)")
                nc.tensor.transpose(
                    tp16[:, n * P:(n + 1) * P],
                    flat[:, h * E:(h + 1) * E],
                    ident_sb,
                )
            nc.vector.tensor_copy(dstT[h][0:E, :], tp16)

        for h in (0, 1):
            for which in ("q", "k"):
                emit_tp(which, h, first=(h == 0 and which == "q"))

        def emit_av(hp, c, ets):
            oc = pout.tile([P, 2, 66], f32, tag="oc", name="oc")
            for a2 in range(c + 1):
                rel = (c - a2) * P
                ch, lo = divmod(rel, 512)
                et = ets[(a2, ch)]

                def avmm(par, start):
                    nc.tensor.matmul(
                        oc[:, par, 0:65],
                        et[:, par, lo:lo + P],
                        v16[:, a2, 2 * hp + par, 0:65],
                        start=start, stop=(a2 == c),
                        skip_group_check=True,
                    )
                if a2 == 0:
                    with tc.tile_critical():
                        avmm(0, True)
                        avmm(1, False)
                else:
                    avmm(0, False)
                    avmm(1, False)
            rz = onorm.tile([P, 2], f32, name="rz")
            nc.vector.reciprocal(rz, oc[:, :, 64])
            for par in (0, 1):
                on = onorm.tile([P, E], f32, name="on")
                nc.vector.tensor_scalar_mul(on, oc[:, par, 0:64],
                                            rz[:, par:par + 1])
                (nc.sync if par == 0 else nc.gpsimd).dma_start(
                    perm_tile(out_d, c, 2 * hp + par), on)

        for hp in range(4):
            ets = {}
            for a in range(NTILE):
                Na = L - P * a
                for off in range(0, Na, 512):
                    w = min(512, Na - off)
                    sc = psc.tile([P, 2, 512], f32, tag="sc", name="sc")
                    for par in (0, 1):
                        h = 2 * hp + par
                        nc.tensor.matmul(
                            sc[:, par, 0:w],
                            kTA[h][0:KF, a * P:(a + 1) * P],
                            qTA[h][0:KF, P * a + off:P * a + off + w],
                            start=True, stop=True,
                        )
                        if off == 0:
                            nc.tensor.matmul(
                                sc[:, par, 0:P],
                                kneg_sb[0:7, :],
                                qnegd_sb[0:7, :],
                                start=False, stop=True,
                                skip_group_check=True,
                            )
                    et = epool.tile([P, 2, 512], f16, tag=f"e{a}_{off // 512}",
                                    name="et")
                    nc.scalar.activation(et[:, :, 0:w], sc[:, :, 0:w], Exp,
                                         scale=float(SCALE))
                    ets[(a, off // 512)] = et
                # A@V one iteration behind: consumes only completed exps,
                # so the PE never queues behind the in-flight ACTIVATE.
                if a >= 1:
                    emit_av(hp, a - 1, ets)
            emit_av(hp, 7, ets)
            # next head-pair's transposes fill the hp boundary
            if hp < 3:
                for par in (0, 1):
                    for which in ("q", "k"):
                        emit_tp(which, 2 * (hp + 1) + par)


def _build():
    if "nc" in _CACHE:
        return _CACHE["nc"], _CACHE["names"]
    from concourse import bacc, mybir
    import concourse.tile as tile

    nc = bacc.Bacc("TRN2", target_bir_lowering=False, debug=False,
                   enable_asserts=False)
    f32, f16 = mybir.dt.float32, mybir.dt.float16
    aps = {}
    aps["queries"] = nc.dram_tensor("queries", [L, H, E], f32,
                                    kind="ExternalInput").ap()
    aps["keys"] = nc.dram_tensor("keys", [L, H, E], f32,
                                 kind="ExternalInput").ap()
    aps["values"] = nc.dram_tensor("values", [L, H, E], f32,
                                   kind="ExternalInput").ap()
    aps["cosF"] = nc.dram_tensor("cosF", [P, NTILE, H, E], f16,
                                 kind="ExternalInput").ap()
    aps["sinF"] = nc.dram_tensor("sinF", [P, NTILE, H, E], f16,
                                 kind="ExternalInput").ap()
    aps["kaugf"] = nc.dram_tensor("kaugf", [H, NAUG, L], f16,
                                  kind="ExternalInput").ap()
    aps["qaugf"] = nc.dram_tensor("qaugf", [NAUG, L], f16,
                                  kind="ExternalInput").ap()
    aps["kneg7"] = nc.dram_tensor("kneg7", [7, P], f16,
                                  kind="ExternalInput").ap()
    aps["qnegd7"] = nc.dram_tensor("qnegd7", [7, P], f16,
                                   kind="ExternalInput").ap()
    aps["ident"] = nc.dram_tensor("ident", [P, P], f16,
                                  kind="ExternalInput").ap()
    aps["out"] = nc.dram_tensor("out", [L, H, E], f32,
                                kind="ExternalOutput").ap()

    with tile.TileContext(nc) as tc:
        _emit(tc, aps)
    nc.compile()
    _CACHE["nc"] = nc
    _CACHE["names"] = {k: v.tensor.name for k, v in aps.items()}
    return nc, _CACHE["names"]


def _in_maps(queries, keys, values, bias_emb):
    cosF, sinF = _rope_tables()
    kaugf, qaugf, kneg7, qnegd7 = _aug_tables(np.asarray(bias_emb, np.float64))
    maps = []
    for b in range(B):
        maps.append({
            "queries": np.ascontiguousarray(np.asarray(queries[b], np.float32)),
            "keys": np.ascontiguousarray(np.asarray(keys[b], np.float32)),
            "values": np.ascontiguousarray(np.asarray(values[b], np.float32)),
            "cosF": cosF, "sinF": sinF,
            "kaugf": kaugf, "qaugf": qaugf,
            "kneg7": kneg7, "qnegd7": qnegd7,
            "ident": np.eye(P, dtype=np.float16),
        })
    return maps


def run(queries, keys, values, bias_emb, trace=False):
    from concourse import bass_utils
    nc, _ = _build()
    maps = _in_maps(queries, keys, values, bias_emb)
    res = bass_utils.run_bass_kernel_spmd(nc, maps, core_ids=list(range(B)),
                                          trace=trace)
    out = np.stack([res.results[b]["out"] for b in range(B)], axis=0)
    return out.astype(np.float32), res


def kernel(queries, keys, values, bias_emb, n_vars=16, group_num=16,
           n_tokens=64, **_ignored):
    assert int(n_vars) == NV and int(n_tokens) == NT
    out, _ = run(np.asarray(queries), np.asarray(keys), np.asarray(values),
                 np.asarray(bias_emb))
    return out



# revision 13
# speedup vs baseline: 1.4193x; 1.4193x over previous
"""Trainium2 Bass kernel for nn_ChannelAttention_56573309224430.

Sharding: data-parallel over batch B=8 across the 8 NeuronCores; each core
computes all H=8 heads for its batch element.

On-core algorithm, in token-major permuted coordinates l' = token*16 + var
(the permutation is just a DMA access pattern on load/store):
  * the mask (s%64 > l%64) becomes block-causal over 16-wide token groups,
    so score tile (a, c) is only computed for a <= c (~half the work);
  * the same/diff-variate bias is 16-periodic: rank-17 rows appended to the
    contraction dim of the QK matmul (K 64 -> 81);
  * the diagonal tile's triangular group mask is rank-7 and 128-periodic:
    seven more rows used ONLY by the diagonal-block matmul (K=88 there);
  * softmax denominator comes from a ones-column appended to V;
  * RoPE (first 32 dims, interleaved pairs) applied in-place on-chip with
    host-precomputed cos/sin tables (in permuted order).
q/k/v stream in as fp16 (host-cast); all matmuls fp16 with fp32 PSUM.
The PE is kept continuously busy (warmup matmuls during the rope phase)
so the HAM clock gate stays at 8/8 = 2.4 GHz.
"""

import numpy as np

B, L, H, E = 8, 1024, 8, 64
NT, NV = 64, 16            # n_tokens, n_vars
P = 128                    # partitions
NTILE = L // P             # 8 tiles of 128 positions
SCALE = 1.0 / np.sqrt(E)   # 0.125
NEG = -30000.0             # large negative, fp16-representable
ROPE_BASE = 10000.0
PH = E // 2                # 32 rotary dims
NF = PH // 2               # 16 frequencies
NAUG = 24                  # 16+1 bias rows + 7 diag-mask rows
KF = E + 17                # off-diagonal contraction depth 81
KD = E + NAUG              # diagonal-block contraction depth 88

_CACHE = {}


# ----------------------------------------------------------------------------
# host-side constant construction
# ----------------------------------------------------------------------------

def _perm_pos():
    """pos(l') for token-major order l' = t*16 + v -> original pos v*64 + t."""
    lp = np.arange(L)
    return (lp % NV) * NT + (lp // NV)


def _rope_tables():
    """Half-width multiplier tables in permuted order, signs folded in.

    cos32[p, n, h, e] = cos(pos*theta_{e//2}) (pair-repeated), e < 32.
    sin32[p, n, h, 2j] = -sin(pos*theta_j), sin32[.., 2j+1] = +sin.
    rope: x[0:32] = x[0:32]*cos32 + swap_pairs(x[0:32])*sin32.
    """
    pos = _perm_pos().astype(np.float64)                       # [1024]
    theta = ROPE_BASE ** (-(np.arange(NF) / NF))               # [16]
    ang = pos[:, None] * theta[None, :]                        # [1024, 16]
    cos32 = np.repeat(np.cos(ang), 2, axis=1)                  # [1024, 32]
    sin32 = np.empty((L, PH))
    sin32[:, 0::2] = -np.sin(ang)
    sin32[:, 1::2] = np.sin(ang)

    def lay(tab):
        t = tab.reshape(NTILE, P, 1, PH).transpose(1, 0, 2, 3)
        return np.ascontiguousarray(
            np.broadcast_to(t, (P, NTILE, H, PH))).astype(np.float16)
    return lay(cos32), lay(sin32)


def _aug_tables(bias_emb):
    """kaug24 [H, 24, 1024], qaug24 [24, 1024] (fp16).

    Rows 0..15: same-variate bias rank-16 product (k: db*(res==r), q: (res==r)).
    Row 16: constant diff bias (k: bdiff, q: 1).
    Rows 17..23: diagonal-block triangular mask, 128-periodic; used only by
    the K=88 diagonal matmul: k: NEG*(g==gi+1), q: (g<=gi), g = (pos%128)//16.
    """
    bdiff = bias_emb[0].astype(np.float64)   # [H]
    bsame = bias_emb[1].astype(np.float64)
    db = bsame - bdiff
    sp = np.arange(L)
    res16 = sp % 16                          # variate id in permuted order
    g = (sp % P) // 16                       # token group within 128-block

    kaug = np.zeros((H, NAUG, L))
    qaug = np.zeros((NAUG, L))
    for r in range(16):
        qaug[r] = (res16 == r)
        for h in range(H):
            kaug[h, r] = db[h] * (res16 == r)
    qaug[16] = 1.0
    for h in range(H):
        kaug[h, 16] = bdiff[h]
    for gi in range(7):
        kaug[:, 17 + gi] = NEG * (g == gi + 1)[None]
        qaug[17 + gi] = (g <= gi)
    return kaug.astype(np.float16), qaug.astype(np.float16)


# ----------------------------------------------------------------------------
# device program
# ----------------------------------------------------------------------------

def _emit(tc, aps):
    import concourse.bass as bass
    from concourse import mybir
    from concourse.tile_rust import add_dep_helper

    nc = tc.nc
    f32 = mybir.dt.float32
    f16 = mybir.dt.float16
    Exp = mybir.ActivationFunctionType.Exp

    def perm_tile(dram_ap, n, h=None, ew=E):
        """Token-major view of DRAM [L, H, ew]: tile n -> [(tr v)=128, h, e]."""
        t = dram_ap.tensor
        off = dram_ap.offset + n * 8 * (H * ew)
        if h is None:
            ap = [[H * ew, 8], [NT * H * ew, NV], [1, H * ew]]
        else:
            off += h * ew
            ap = [[H * ew, 8], [NT * H * ew, NV], [1, E]]
        return bass.AP(tensor=t, offset=off, ap=ap)

    q_d, k_d, v_d, out_d = aps["queries"], aps["keys"], aps["values"], aps["out"]

    from contextlib import ExitStack
    with ExitStack() as ctx:
        consts = ctx.enter_context(tc.tile_pool(name="consts", bufs=1))
        tmpp = ctx.enter_context(tc.tile_pool(name="tmpp", bufs=4))
        epool = ctx.enter_context(tc.tile_pool(name="epool", bufs=2))
        onorm = ctx.enter_context(tc.tile_pool(name="onorm", bufs=8))

        # ---- constants ----
        cos_sb = consts.tile([P, NTILE, H, PH], f16)
        nc.scalar.dma_start(cos_sb, aps["cos32"])
        sin_sb = consts.tile([P, NTILE, H, PH], f16)
        nc.scalar.dma_start(sin_sb, aps["sin32"])
        ident_sb = consts.tile([P, P], f16)
        nc.sync.dma_start(ident_sb, aps["ident"])

        # fused q/k tiles: rows 0:64 = transposed rope'd data, 64:88 = aug rows
        qTA = [consts.tile([KD, L], f16, name=f"qTA{h}") for h in range(H)]
        kTA = [consts.tile([KD, L], f16, name=f"kTA{h}") for h in range(H)]
        for h in range(H):
            nc.scalar.dma_start(kTA[h][E:KD, :], aps["kaug24"][h])
            nc.scalar.dma_start(qTA[h][E:KD, :], aps["qaug24"])

        # V' = [V | 1 | pad] loaded directly as fp16 (ones column host-padded)
        v16 = consts.tile([P, NTILE, H, 66], f16)
        for n in range(NTILE):
            nc.gpsimd.dma_start(v16[:, n], perm_tile(v_d, n, ew=66))

        # ---- staged q/k (fp16) + in-place rope (q on DVE, k on GPSIMD) ----
        qstage = consts.tile([P, NTILE, H, E], f16)
        kstage = consts.tile([P, NTILE, H, E], f16)
        for n in range(NTILE):
            nc.sync.dma_start(qstage[:, n], perm_tile(q_d, n))
            nc.scalar.dma_start(kstage[:, n], perm_tile(k_d, n))
        for n in range(NTILE):
            for stage, eng, tag in ((qstage, nc.vector, "q"), (kstage, nc.gpsimd, "k")):
                x = stage[:, n, :, 0:PH]
                xp = x.rearrange("p h (j i) -> p h j i", i=2)
                m1 = tmpp.tile([P, H, PH], f16, name=f"m1{tag}")
                m2 = tmpp.tile([P, H, PH], f16, name=f"m2{tag}")
                m2p = m2.rearrange("p h (j i) -> p h j i", i=2)
                sinp = sin_sb[:, n].rearrange("p h (j i) -> p h j i", i=2)
                eng.tensor_mul(m1, x, cos_sb[:, n])
                eng.tensor_mul(m2p[:, :, :, 0], xp[:, :, :, 1], sinp[:, :, :, 0])
                eng.tensor_mul(m2p[:, :, :, 1], xp[:, :, :, 0], sinp[:, :, :, 1])
                eng.tensor_add(x, m1, m2)

        # ---- attention ----
        psc = ctx.enter_context(tc.tile_pool(name="psc", bufs=2, space="PSUM"))
        pout = ctx.enter_context(tc.tile_pool(name="pout", bufs=2, space="PSUM"))
        ptp = ctx.enter_context(tc.tile_pool(name="ptp", bufs=2, space="PSUM"))

        # PE warm-up: real matmuls (transpose-mode does not count for the HAM
        # activity window); sized to span the rope phase so the PE is warm
        # (8/8 clock) when the first QK matmul issues.
        wsb = consts.tile([P, 512], f16, name="wsb")
        nc.gpsimd.memset(wsb, 0.0)
        last_pe = None

        def warm(count):
            nonlocal last_pe
            warm_ps = pout.tile([P, 512], f32, tag="oc", name="warm_ps")
            for _ in range(count):
                g = nc.tensor.matmul(warm_ps, wsb[0:P, 0:P], wsb,
                                     start=True, stop=True,
                                     skip_group_check=True)
                if last_pe is not None:
                    add_dep_helper(g.ins, last_pe.ins, sync=False,
                                   reason="pe order")
                last_pe = g

        def chain(g, force=True):
            # PE-queue ordering hints. Never chain into/out of tile_critical
            # blocks (the scheduler deadlocks); those calls pass force=False
            # and do not advance the chain head.
            nonlocal last_pe
            if not force:
                return
            if last_pe is not None:
                add_dep_helper(g.ins, last_pe.ins, sync=False, reason="pe order")
            last_pe = g

        # transposes: per (side, head-pair): 8 two-head [128,128] transposes
        # into one PSUM tile, then 2 copies [64, 1024] to SBUF.
        def emit_tp(which, hp, copy_engines):
            stage = qstage if which == "q" else kstage
            dstT = qTA if which == "q" else kTA
            tp16 = ptp.tile([P, L], f16, tag="tp", name="tp16")
            for n in range(NTILE):
                flat = stage[:, n].rearrange("p h e -> p (h e)")
                g = nc.tensor.transpose(
                    tp16[:, n * P:(n + 1) * P],
                    flat[:, 2 * hp * E:(2 * hp + 2) * E],
                    ident_sb,
                )
                chain(g)
            for par, eng in zip((0, 1), copy_engines):
                dst = dstT[2 * hp + par][0:E, :]
                src = tp16[par * E:(par + 1) * E, :]
                if eng is nc.scalar:
                    eng.copy(dst, src)
                else:
                    eng.tensor_copy(dst, src)

        # warmup spanning the rope phase, hp0 transposes interleaved
        warm(10)
        emit_tp("q", 0, (nc.scalar, nc.scalar))
        warm(4)
        emit_tp("k", 0, (nc.scalar, nc.scalar))
        warm(4)

        def emit_av(hp, c, ets):
            oc = pout.tile([P, 2, 66], f32, tag="oc", name="oc")
            for a2 in range(c + 1):
                rel = (c - a2) * P
                ch, lo = divmod(rel, 512)
                et = ets[(a2, ch)]

                def avmm(par, start, force=True):
                    g = nc.tensor.matmul(
                        oc[:, par, 0:65],
                        et[:, par, lo:lo + P],
                        v16[:, a2, 2 * hp + par, 0:65],
                        start=start, stop=(a2 == c),
                        skip_group_check=True,
                    )
                    chain(g, force=force)
                if a2 == 0:
                    with tc.tile_critical():
                        avmm(0, True, force=False)
                        avmm(1, False, force=False)
                else:
                    avmm(0, False)
                    avmm(1, False)
            rz = onorm.tile([P, 2], f32, name="rz")
            nc.vector.reciprocal(rz, oc[:, :, 64])
            for par in (0, 1):
                on = onorm.tile([P, E], f16, name="on")
                nc.vector.tensor_scalar_mul(on, oc[:, par, 0:64],
                                            rz[:, par:par + 1])
                (nc.sync if par == 0 else nc.gpsimd).dma_start(
                    perm_tile(out_d, c, 2 * hp + par), on)

        for hp in range(4):
            ets = {}
            for a in range(NTILE):
                Na = L - P * a
                # chunk 0: diagonal block (K=88) + next 384 cols (K=81)
                sc = psc.tile([P, 2, 512], f32, tag="sc", name="sc")
                w0 = min(512, Na)
                for par in (0, 1):
                    h = 2 * hp + par
                    g = nc.tensor.matmul(
                        sc[:, par, 0:P],
                        kTA[h][0:KD, a * P:(a + 1) * P],
                        qTA[h][0:KD, a * P:(a + 1) * P],
                        start=True, stop=True,
                    )
                    chain(g)
                    if w0 > P:
                        g = nc.tensor.matmul(
                            sc[:, par, P:w0],
                            kTA[h][0:KF, a * P:(a + 1) * P],
                            qTA[h][0:KF, a * P + P:a * P + w0],
                            start=True, stop=True,
                            skip_group_check=True,
                        )
                        chain(g)
                et = epool.tile([P, 2, 512], f16, tag=f"e{a}_0", name="et")
                nc.scalar.activation(et[:, :, 0:w0], sc[:, :, 0:w0], Exp,
                                     scale=float(SCALE))
                ets[(a, 0)] = et
                if Na > 512:
                    w1 = Na - 512
                    sc1 = psc.tile([P, 2, 512], f32, tag="sc", name="sc1")
                    for par in (0, 1):
                        h = 2 * hp + par
                        g = nc.tensor.matmul(
                            sc1[:, par, 0:w1],
                            kTA[h][0:KF, a * P:(a + 1) * P],
                            qTA[h][0:KF, a * P + 512:a * P + 512 + w1],
                            start=True, stop=True,
                        )
                        chain(g)
                    et1 = epool.tile([P, 2, 512], f16, tag=f"e{a}_1", name="et1")
                    nc.scalar.activation(et1[:, :, 0:w1], sc1[:, :, 0:w1], Exp,
                                         scale=float(SCALE))
                    ets[(a, 1)] = et1
                # A@V two iterations behind: consumes only completed exps
                if a >= 2:
                    emit_av(hp, a - 2, ets)
            emit_av(hp, 6, ets)
            # next head-pair's transposes fill the hp boundary
            if hp < 3:
                emit_tp("q", hp + 1, (nc.vector, nc.vector))
                emit_tp("k", hp + 1, (nc.vector, nc.vector))
            emit_av(hp, 7, ets)


def _build():
    if "nc" in _CACHE:
        return _CACHE["nc"], _CACHE["names"]
    from concourse import bacc, mybir
    import concourse.tile as tile

    nc = bacc.Bacc("TRN2", target_bir_lowering=False, debug=False,
                   enable_asserts=False)
    f16 = mybir.dt.float16
    aps = {}
    for name in ("queries", "keys"):
        aps[name] = nc.dram_tensor(name, [L, H, E], f16,
                                   kind="ExternalInput").ap()
    aps["values"] = nc.dram_tensor("values", [L, H, 66], f16,
                                   kind="ExternalInput").ap()
    aps["cos32"] = nc.dram_tensor("cos32", [P, NTILE, H, PH], f16,
                                  kind="ExternalInput").ap()
    aps["sin32"] = nc.dram_tensor("sin32", [P, NTILE, H, PH], f16,
                                  kind="ExternalInput").ap()
    aps["kaug24"] = nc.dram_tensor("kaug24", [H, NAUG, L], f16,
                                   kind="ExternalInput").ap()
    aps["qaug24"] = nc.dram_tensor("qaug24", [NAUG, L], f16,
                                   kind="ExternalInput").ap()
    aps["ident"] = nc.dram_tensor("ident", [P, P], f16,
                                  kind="ExternalInput").ap()
    aps["out"] = nc.dram_tensor("out", [L, H, E], f16,
                                kind="ExternalOutput").ap()

    with tile.TileContext(nc) as tc:
        _emit(tc, aps)
    nc.compile()
    _CACHE["nc"] = nc
    _CACHE["names"] = {k: v.tensor.name for k, v in aps.items()}
    return nc, _CACHE["names"]


def _in_maps(queries, keys, values, bias_emb):
    cos32, sin32 = _rope_tables()
    kaug24, qaug24 = _aug_tables(np.asarray(bias_emb, np.float64))
    v66 = np.zeros((B, L, H, 66), np.float16)
    v66[:, :, :, 0:64] = np.asarray(values, np.float16)
    v66[:, :, :, 64] = 1.0
    maps = []
    for b in range(B):
        maps.append({
            "queries": np.ascontiguousarray(np.asarray(queries[b], np.float16)),
            "keys": np.ascontiguousarray(np.asarray(keys[b], np.float16)),
            "values": np.ascontiguousarray(v66[b]),
            "cos32": cos32, "sin32": sin32,
            "kaug24": kaug24, "qaug24": qaug24,
            "ident": np.eye(P, dtype=np.float16),
        })
    return maps


def run(queries, keys, values, bias_emb, trace=False):
    from concourse import bass_utils
    nc, _ = _build()
    maps = _in_maps(queries, keys, values, bias_emb)
    res = bass_utils.run_bass_kernel_spmd(nc, maps, core_ids=list(range(B)),
                                          trace=trace)
    out = np.stack([res.results[b]["out"] for b in range(B)], axis=0)
    return out.astype(np.float32), res


def kernel(queries, keys, values, bias_emb, n_vars=16, group_num=16,
           n_tokens=64, **_ignored):
    assert int(n_vars) == NV and int(n_tokens) == NT
    out, _ = run(np.asarray(queries), np.asarray(keys), np.asarray(values),
                 np.asarray(bias_emb))
    return out
